# revision 31
# baseline (speedup 1.0000x reference)
"""Bahdanau additive attention kernel for Trainium2 (8 NeuronCores, SPMD).

Problem (hardcoded): B=32, Tq=4, S=2048, H=1024, 2H=2048, fp32 inputs.
  q  = query[:, -1, :]                      [B, H]
  k  = transpose(keys, (1, 0, 2))           [B, S, 2H]
  wq = q @ Wa_w.T + Wa_b                    [B, H]
  uk = k @ Ua_w.T + Ua_b                    [B, S, H]
  sc = tanh(wq[:, None, :] + uk) @ Va_w.T   [B, S]   (+ Va_b, which softmax cancels)
  w  = softmax(sc, axis=-1)                 [B, S]
  ctx = w @ k                               [B, 2H]
  returns (ctx [B,1,2H], w [B,1,S])

Sharding: data-parallel over batch. 8 cores x 4 batches each; weights
replicated; no cross-core communication.

Layout strategy: the host pre-arranges the small weight tensors into the
layouts the PE needs (Ua/Wa transposed so the contraction dim lands on
partitions, q/Va as column panels) and pre-casts them to bf16 -- pure
permutation/rounding marshalling, all model arithmetic stays on device.

Per-core dataflow:
  - keys are cast f32->bf16 chunk-by-chunk with single DRAM->DRAM SWDGE
    instructions (gpsimd queue), then read back transposed via the DMA
    xbar on the sync queue as [d=128, s=1024] group tiles, prefetched a
    group ahead.  No SBUF copy of the natural-layout keys is kept.
  - ukT tiles [h=128, s=512] accumulate in PSUM (bf16 matmuls, fp32
    accum); ScalarE applies tanh(. + bias[h]) where
    bias = wq[b] + Wa_b + Ua_b is folded per-partition.
  - scores via PE with Va columns as the 1-wide stationary operand; exp
    on ScalarE (no max subtraction; scores are O(1)) with free-dim
    accumulate for the softmax denominator.
  - context on the Vector engine: the exp-score row is broadcast across
    partitions with a 1-row PE matmul into PSUM, then per d-strip a
    fused tensor_tensor_reduce multiplies the transposed keys tile by it
    and reduces along s, chaining the per-chunk partials through the
    reduce's initial-value operand.  ctxT [d=128, SD] is transposed back
    on PE and normalized at batch end.
  - outputs DMA'd on the sync queue.
"""

import numpy as np

B, TQ, S, H = 32, 4, 2048, 1024
D2 = 2 * H
NCORES = 8
BPC = B // NCORES  # batches per core

_CACHE = {}


def _build(s=S, h=H, bpc=BPC, schunk=512):
    """Build the per-core Bass module. Parameterized so a scaled-down config
    can run in CoreSim; the shipped kernel uses the defaults."""
    from contextlib import ExitStack

    import concourse.bacc as bacc
    import concourse.bass as bass
    import concourse.mybir as mybir
    import concourse.tile as tile
    from concourse.masks import make_identity

    fp32 = mybir.dt.float32
    bf16 = mybir.dt.bfloat16
    AF = mybir.ActivationFunctionType
    ALU = mybir.AluOpType
    d2 = 2 * h
    SD = d2 // 128        # contraction strips for uk (d on partitions)
    SM = h // 128         # h tiles (uk output partitions / Va strips)
    SJ = h // 128         # contraction strips for wq
    NCH = s // schunk     # score chunks per batch
    NPOS = bpc * NCH      # total chunk positions
    LG = 2 if NCH % 2 == 0 else 1  # chunks per kT group

    nc = bacc.Bacc(
        "TRN2", target_bir_lowering=False, enable_partition_id=False
    )

    qt_in = nc.dram_tensor("qt", [128, SJ * bpc], bf16, kind="ExternalInput").ap()
    keys_in = nc.dram_tensor("keys", [s, bpc, d2], fp32, kind="ExternalInput").ap()
    wat_in = nc.dram_tensor("wat", [h, h], bf16, kind="ExternalInput").ap()
    uat_in = nc.dram_tensor("uat", [d2, h], bf16, kind="ExternalInput").ap()
    wab_in = nc.dram_tensor("wab", [1, h], bf16, kind="ExternalInput").ap()
    uab_in = nc.dram_tensor("uab", [1, h], bf16, kind="ExternalInput").ap()
    vac_in = nc.dram_tensor("vac", [128, SM], bf16, kind="ExternalInput").ap()
    ctx_out = nc.dram_tensor("ctx", [bpc, d2], fp32, kind="ExternalOutput").ap()
    w_out = nc.dram_tensor("wts", [bpc, s], fp32, kind="ExternalOutput").ap()

    with tile.TileContext(nc) as tc:
        with ExitStack() as ctx:
            consts = ctx.enter_context(tc.tile_pool(name="consts", bufs=1))
            dram_kn = ctx.enter_context(
                tc.tile_pool(name="dram_kn", bufs=3, space="DRAM")
            )
            kstage = ctx.enter_context(tc.tile_pool(name="kstage", bufs=8))
            ktp = ctx.enter_context(tc.tile_pool(name="ktp", bufs=2 * SD + 8))
            tp = ctx.enter_context(tc.tile_pool(name="tp", bufs=SM + 1))
            rows = ctx.enter_context(tc.tile_pool(name="rows", bufs=2))
            acc1 = ctx.enter_context(tc.tile_pool(name="acc1", bufs=2))
            ps_setup = ctx.enter_context(
                tc.tile_pool(name="ps_setup", bufs=1, space="PSUM")
            )
            ps_uk = ctx.enter_context(tc.tile_pool(name="ps_uk", bufs=3, space="PSUM"))
            ps_sc = ctx.enter_context(tc.tile_pool(name="ps_sc", bufs=2, space="PSUM"))
            ps_eb = ctx.enter_context(tc.tile_pool(name="ps_eb", bufs=2, space="PSUM"))

            # ---------------- keys pipeline helpers ----------------
            knats = {}
            pending_strips = {}
            pending_kts = {}

            SPC = schunk // 128

            def issue_cast(b, c):
                # stage this chunk's keys rows f32->bf16 into the per-batch
                # DRAM scratch via transient SBUF strips, all on the gpsimd
                # queue: each store follows its producer load in FIFO order,
                # so staging is fully self-paced and never blocks another
                # engine's queue
                if b not in knats:
                    knats[b] = dram_kn.tile(
                        [s, d2], bf16, tag="knat", name=f"knat_b{b}"
                    )
                knat = knats[b]
                strips = []
                for i in range(SPC):
                    si = c * SPC + i
                    ks = kstage.tile(
                        [128, d2], bf16, tag="ks", name=f"ks_{b}_{si}"
                    )
                    nc.gpsimd.dma_start(
                        out=ks, in_=keys_in[si * 128 : (si + 1) * 128, b, :]
                    )
                    strips.append(ks)
                for i, ks in enumerate(strips):
                    si = c * SPC + i
                    nc.gpsimd.dma_start(
                        out=knat[si * 128 : (si + 1) * 128, :], in_=ks
                    )

            def issue_kts(b, g, waves=1):
                # transposed [d=128, s=LG*schunk] tiles via the DMA xbar on
                # the sync queue; one xbar per d covering LG chunks (the
                # ~1.2us fixed cost per xbar makes fewer/bigger transposes
                # cheaper). waves=LG splits each tile fill into per-chunk
                # xbars so the first chunk's matmuls can start sooner.
                knat = knats[b]
                kts = []
                for d in range(SD):
                    kt = ktp.tile(
                        [128, LG * schunk], bf16, tag="kt", name=f"kt_{b}_{g}_{d}"
                    )
                    kts.append(kt)
                r0 = g * LG * schunk
                rows_w = LG * schunk // waves
                for w in range(waves):
                    for d in range(SD):
                        nc.sync.dma_start(
                            out=kts[d][:, w * rows_w : (w + 1) * rows_w],
                            in_=knat[
                                r0 + w * rows_w : r0 + (w + 1) * rows_w,
                                d * 128 : (d + 1) * 128,
                            ],
                            transpose=True,
                        )
                pending_kts[(b, g)] = kts

            # ---------------- one-time setup ----------------
            ident_f32 = consts.tile([128, 128], fp32)
            make_identity(nc, ident_f32)

            # combined additive bias row (Wa_b + Ua_b) via accumulate-DMA;
            # issued first on gpsimd (tiny) since it gates chunk 0's tanh
            comb_bf = consts.tile([1, h], bf16)
            nc.gpsimd.dma_start(out=comb_bf, in_=uab_in)
            nc.gpsimd.dma_start(
                out=comb_bf, in_=wab_in, accum_op=ALU.add
            )

            # keys pipeline for the first chunks starts immediately
            issue_cast(0, 0)
            if NCH > 1:
                issue_cast(0, 1)
            issue_kts(0, 0, waves=LG)
            if NCH > 2:
                issue_cast(0, 2)

            # weight panels arrive pre-transposed/pre-cast; plain HWDGE
            # loads on the scalar queue (no SWDGE cast needed)
            qT = consts.tile([128, SJ, bpc], bf16)
            nc.scalar.dma_start(out=qT, in_=qt_in)
            va_cols = consts.tile([128, SM], bf16)
            nc.scalar.dma_start(out=va_cols, in_=vac_in)
            waT = consts.tile([128, SJ, h], bf16)
            for j in range(SJ):
                nc.scalar.dma_start(
                    out=waT[:, j, :], in_=wat_in[j * 128 : (j + 1) * 128, :]
                )
            uaT = consts.tile([128, SD, h], bf16)
            for d in range(SD):
                nc.scalar.dma_start(
                    out=uaT[:, d, :], in_=uat_in[d * 128 : (d + 1) * 128, :]
                )

            ones_bf = consts.tile([1, 128], bf16)
            nc.vector.memset(ones_bf, 1.0)
            ones_f32 = consts.tile([1, 128], fp32)
            nc.vector.memset(ones_f32, 1.0)

            # bias_cols[:, m, b] = (Wa q_b)[128m:128m+128] + Wa_b + Ua_b  (fp32)
            bias_cols = consts.tile([128, SM, bpc], fp32)
            for m in range(SM):
                pw = ps_setup.tile([128, bpc], fp32, tag="setup")
                for j in range(SJ):
                    nc.tensor.matmul(
                        out=pw,
                        lhsT=waT[:, j, m * 128 : (m + 1) * 128],
                        rhs=qT[:, j, :],
                        start=(j == 0),
                        stop=False,
                    )
                nc.tensor.matmul(
                    out=pw,
                    lhsT=comb_bf[:1, m * 128 : (m + 1) * 128],
                    rhs=ones_bf[:, :bpc],
                    start=False,
                    stop=True,
                )
                nc.vector.tensor_copy(out=bias_cols[:, m, :], in_=pw)

            # ---------------- main loop over batches ----------------
            kts_group = None
            deferred_tail = []
            for b in range(bpc):
                exp_row = rows.tile([1, s], fp32, tag="exp_row", bufs=1)
                tparts = rows.tile([1, NCH], fp32, tag="tparts")
                ctxT = acc1.tile([128, SD], fp32, tag="ctxT")
                for c in range(NCH):
                    pos = b * NCH + c
                    # prefetch the staging cast three chunks ahead; at each
                    # group boundary issue the next group's xbars (they
                    # sem-wait on the casts and land a chunk before use)
                    if pos + 3 < NPOS:
                        nxt = pos + 3
                        issue_cast(nxt // NCH, nxt % NCH)
                    if c % LG == 0:
                        if pos + LG < NPOS:
                            ngp = pos + LG
                            issue_kts(ngp // NCH, (ngp % NCH) // LG)
                        kts_group = pending_kts.pop((b, c // LG))
                    kts = [
                        kt[:, (c % LG) * schunk : (c % LG + 1) * schunk]
                        for kt in kts_group
                    ]
                    if c == 1 and deferred_tail:
                        deferred_tail.pop(0)()
                    # ukT tiles + tanh; score matmuls are deferred until all
                    # tanh tiles exist so the in-order PE queue never waits
                    # on the Scalar engine mid-chunk
                    psc = ps_sc.tile([1, schunk], fp32, tag="psc")
                    ts_list = []
                    for m in range(SM):
                        puk = ps_uk.tile([128, schunk], fp32, tag="puk")
                        for d in range(SD):
                            nc.tensor.matmul(
                                out=puk,
                                lhsT=uaT[:, d, m * 128 : (m + 1) * 128],
                                rhs=kts[d],
                                start=(d == 0),
                                stop=(d == SD - 1),
                            )
                        t_sb = tp.tile([128, schunk], bf16, tag="t")
                        nc.scalar.activation(
                            out=t_sb,
                            in_=puk,
                            func=AF.Tanh,
                            bias=bias_cols[:, m, b : b + 1],
                            scale=1.0,
                        )
                        ts_list.append(t_sb)
                    for m in range(SM):
                        nc.tensor.matmul(
                            out=psc,
                            lhsT=va_cols[:, m : m + 1],
                            rhs=ts_list[m],
                            start=(m == 0),
                            stop=(m == SM - 1),
                        )
                    # exp row chunk (no max subtraction; scores are O(1)) and
                    # the chunk's softmax partial sum
                    nc.scalar.activation(
                        out=exp_row[:, c * schunk : (c + 1) * schunk],
                        in_=psc,
                        func=AF.Exp,
                        accum_out=tparts[:, c : c + 1],
                    )
                    # context on DVE: broadcast the exp row across partitions
                    # (1-row PE matmul into PSUM), then fused multiply+reduce
                    # against each transposed-keys strip, chaining the
                    # partials through the reduce's initial value
                    ebc = ps_eb.tile([128, schunk], fp32, tag="ebc")
                    nc.tensor.matmul(
                        out=ebc,
                        lhsT=ones_f32,
                        rhs=exp_row[:1, c * schunk : (c + 1) * schunk],
                        start=True,
                        stop=True,
                    )
                    sub = c % LG
                    if sub == 0:
                        ebc_sb = acc1.tile(
                            [128, LG * schunk],
                            bf16,
                            tag="ebc_sb",
                            name=f"ebcsb_{b}_{c // LG}",
                        )
                    nc.scalar.activation(
                        out=ebc_sb[:, sub * schunk : (sub + 1) * schunk],
                        in_=ebc,
                        func=AF.Copy,
                    )
                    if sub == LG - 1:
                        # per-group DVE burst: multiply each transposed-keys
                        # group tile by the exp rows and reduce along s; the
                        # mult->reduce->add chain stays on the vector queue
                        # so only the ebc copy crosses engines
                        for d in range(SD):
                            prod = acc1.tile(
                                [128, LG * schunk],
                                bf16,
                                tag="prod",
                                name=f"prod_{b}_{c}_{d}",
                            )
                            nc.vector.tensor_tensor(
                                out=prod, in0=kts_group[d], in1=ebc_sb, op=ALU.mult
                            )
                            part = acc1.tile(
                                [128, 1], fp32, tag="part", name=f"part_{b}_{c}_{d}"
                            )
                            nc.vector.reduce_sum(
                                out=part, in_=prod, axis=mybir.AxisListType.X
                            )
                            if c < LG:
                                nc.vector.tensor_copy(
                                    out=ctxT[:, d : d + 1], in_=part
                                )
                            else:
                                nc.vector.tensor_add(
                                    out=ctxT[:, d : d + 1],
                                    in0=ctxT[:, d : d + 1],
                                    in1=part,
                                )
                # softmax denominator; normalize weights, write them out
                tsum = rows.tile([1, 1], fp32, tag="tsum")
                nc.vector.reduce_sum(
                    out=tsum, in_=tparts, axis=mybir.AxisListType.X
                )
                invt = rows.tile([1, 1], fp32, tag="invt")
                nc.vector.reciprocal(out=invt, in_=tsum)
                nc.vector.tensor_scalar_mul(out=exp_row, in0=exp_row, scalar1=invt)
                nc.gpsimd.dma_start(out=w_out[b : b + 1, :], in_=exp_row)

                def ctx_tail(b=b, ctxT=ctxT, invt=invt):
                    # ctxT [d2-part, SD] -> rows [SD, 128] on PE, scaled by
                    # 1/T (replicated to SD partitions via a tiny fp32
                    # matmul).  Emitted one chunk into the NEXT batch so the
                    # in-order PE queue never waits on the final DVE context
                    # burst at the batch boundary.
                    pinv = ps_setup.tile([SD, 1], fp32, tag="setup")
                    nc.tensor.matmul(
                        out=pinv,
                        lhsT=ones_f32[:1, :SD],
                        rhs=invt,
                        start=True,
                        stop=True,
                    )
                    inv_col = rows.tile([SD, 1], fp32, tag="inv_col")
                    nc.vector.tensor_copy(out=inv_col, in_=pinv)
                    pctx = ps_setup.tile([SD, 128], fp32, tag="setup")
                    nc.tensor.transpose(
                        out=pctx, in_=ctxT, identity=ident_f32
                    )
                    ctx_rows = acc1.tile([SD, 128], fp32, tag="ctx_rows")
                    nc.vector.tensor_scalar_mul(
                        out=ctx_rows, in0=pctx, scalar1=inv_col
                    )
                    nc.gpsimd.dma_start(out=ctx_out[b : b + 1, :], in_=ctx_rows)

                deferred_tail.append(ctx_tail)
            while deferred_tail:
                deferred_tail.pop(0)()

    nc.compile()
    return nc


def _get_nc():
    if "nc" not in _CACHE:
        _CACHE["nc"] = _build()
    return _CACHE["nc"]


def _make_in_maps(inputs):
    import ml_dtypes

    bf16 = ml_dtypes.bfloat16
    SJ = H // 128
    SM = H // 128

    q_last = np.asarray(inputs["query"], dtype=np.float32)[:, -1, :]  # [B, H]
    keys = np.asarray(inputs["keys"], dtype=np.float32)  # [S, B, 2H]
    # weight panels: pre-transpose/pre-arrange + bf16 cast (layout
    # marshalling only; same rounding the device DMA cast applied)
    uaT = np.ascontiguousarray(
        np.asarray(inputs["Ua_w"], dtype=np.float32).T
    ).astype(bf16)  # [2H, H]
    waT = np.ascontiguousarray(
        np.asarray(inputs["Wa_w"], dtype=np.float32).T
    ).astype(bf16)  # [H, H]
    vac = np.ascontiguousarray(
        np.asarray(inputs["Va_w"], dtype=np.float32).reshape(SM, 128).T
    ).astype(bf16)  # [128, SM]
    uab = np.asarray(inputs["Ua_b"], dtype=np.float32).reshape(1, H).astype(bf16)
    wab = np.asarray(inputs["Wa_b"], dtype=np.float32).reshape(1, H).astype(bf16)

    in_maps = []
    for cidx in range(NCORES):
        b0 = cidx * BPC
        qt = np.ascontiguousarray(
            q_last[b0 : b0 + BPC]
            .T.reshape(SJ, 128, BPC)
            .transpose(1, 0, 2)
            .reshape(128, SJ * BPC)
        ).astype(bf16)
        in_maps.append(
            {
                "qt": qt,
                "keys": np.ascontiguousarray(keys[:, b0 : b0 + BPC, :]),
                "wat": waT,
                "uat": uaT,
                "wab": wab,
                "uab": uab,
                "vac": vac,
            }
        )
    return in_maps


def run(inputs, trace=False, **kwargs):
    """Run on all 8 cores; returns ((context, weights), BassKernelResults)."""
    from concourse.bass_utils import run_bass_kernel_spmd

    nc = _get_nc()
    in_maps = _make_in_maps(inputs)
    res = run_bass_kernel_spmd(
        nc, in_maps, core_ids=list(range(NCORES)), trace=trace, **kwargs
    )
    context = np.empty((B, 1, D2), dtype=np.float32)
    weights = np.empty((B, 1, S), dtype=np.float32)
    for c in range(NCORES):
        b0 = c * BPC
        context[b0 : b0 + BPC, 0, :] = res.results[c]["ctx"]
        weights[b0 : b0 + BPC, 0, :] = res.results[c]["wts"]
    return (context, weights), res


def kernel(**inputs):
    out, _ = run(inputs)
    return out


# revision 34
# speedup vs baseline: 1.0431x; 1.0431x over previous
"""Bahdanau additive attention kernel for Trainium2 (8 NeuronCores, SPMD).

Problem (hardcoded): B=32, Tq=4, S=2048, H=1024, 2H=2048, fp32 inputs.
  q  = query[:, -1, :]                      [B, H]
  k  = transpose(keys, (1, 0, 2))           [B, S, 2H]
  wq = q @ Wa_w.T + Wa_b                    [B, H]
  uk = k @ Ua_w.T + Ua_b                    [B, S, H]
  sc = tanh(wq[:, None, :] + uk) @ Va_w.T   [B, S]   (+ Va_b, which softmax cancels)
  w  = softmax(sc, axis=-1)                 [B, S]
  ctx = w @ k                               [B, 2H]
  returns (ctx [B,1,2H], w [B,1,S])

Sharding: data-parallel over batch. 8 cores x 4 batches each; weights
replicated; no cross-core communication.

Layout strategy: the host pre-arranges the small weight tensors into the
layouts the PE needs (Ua/Wa transposed so the contraction dim lands on
partitions, q/Va as column panels) and pre-casts them to bf16 -- pure
permutation/rounding marshalling, all model arithmetic stays on device.

Per-core dataflow:
  - keys are cast f32->bf16 chunk-by-chunk with single DRAM->DRAM SWDGE
    instructions (gpsimd queue), then read back transposed via the DMA
    xbar on the sync queue as [d=128, s=1024] group tiles, prefetched a
    group ahead.  No SBUF copy of the natural-layout keys is kept.
  - ukT tiles [h=128, s=512] accumulate in PSUM (bf16 matmuls, fp32
    accum); ScalarE applies tanh(. + bias[h]) where
    bias = wq[b] + Wa_b + Ua_b is folded per-partition.
  - scores via PE with Va columns as the 1-wide stationary operand; exp
    on ScalarE (no max subtraction; scores are O(1)) with free-dim
    accumulate for the softmax denominator.
  - context on the Vector engine: the exp-score row is broadcast across
    partitions with a 1-row PE matmul into PSUM, then per d-strip a
    fused tensor_tensor_reduce multiplies the transposed keys tile by it
    and reduces along s, chaining the per-chunk partials through the
    reduce's initial-value operand.  ctxT [d=128, SD] is transposed back
    on PE and normalized at batch end.
  - outputs DMA'd on the sync queue.
"""

import numpy as np

B, TQ, S, H = 32, 4, 2048, 1024
D2 = 2 * H
NCORES = 8
BPC = B // NCORES  # batches per core

_CACHE = {}


def _build(s=S, h=H, bpc=BPC, schunk=512):
    """Build the per-core Bass module. Parameterized so a scaled-down config
    can run in CoreSim; the shipped kernel uses the defaults."""
    from contextlib import ExitStack

    import concourse.bacc as bacc
    import concourse.bass as bass
    import concourse.mybir as mybir
    import concourse.tile as tile
    from concourse.masks import make_identity

    fp32 = mybir.dt.float32
    bf16 = mybir.dt.bfloat16
    AF = mybir.ActivationFunctionType
    ALU = mybir.AluOpType
    d2 = 2 * h
    SD = d2 // 128        # contraction strips for uk (d on partitions)
    SM = h // 128         # h tiles (uk output partitions / Va strips)
    SJ = h // 128         # contraction strips for wq
    NCH = s // schunk     # score chunks per batch
    NPOS = bpc * NCH      # total chunk positions
    LG = 2 if NCH % 2 == 0 else 1  # chunks per kT group

    nc = bacc.Bacc(
        "TRN2", target_bir_lowering=False, enable_partition_id=False
    )

    qt_in = nc.dram_tensor("qt", [128, SJ * bpc], bf16, kind="ExternalInput").ap()
    keys_in = nc.dram_tensor("keys", [s, bpc, d2], fp32, kind="ExternalInput").ap()
    wat_in = nc.dram_tensor("wat", [h, h], bf16, kind="ExternalInput").ap()
    uat_in = nc.dram_tensor("uat", [d2, h], bf16, kind="ExternalInput").ap()
    wab_in = nc.dram_tensor("wab", [1, h], bf16, kind="ExternalInput").ap()
    uab_in = nc.dram_tensor("uab", [1, h], bf16, kind="ExternalInput").ap()
    vac_in = nc.dram_tensor("vac", [128, SM], bf16, kind="ExternalInput").ap()
    ctx_out = nc.dram_tensor("ctx", [bpc, d2], fp32, kind="ExternalOutput").ap()
    w_out = nc.dram_tensor("wts", [bpc, s], fp32, kind="ExternalOutput").ap()

    with tile.TileContext(nc) as tc:
        with ExitStack() as ctx:
            consts = ctx.enter_context(tc.tile_pool(name="consts", bufs=1))
            dram_kn = ctx.enter_context(
                tc.tile_pool(name="dram_kn", bufs=3, space="DRAM")
            )
            kstage = ctx.enter_context(tc.tile_pool(name="kstage", bufs=10))
            ktp = ctx.enter_context(tc.tile_pool(name="ktp", bufs=2 * SD + 8))
            tp = ctx.enter_context(tc.tile_pool(name="tp", bufs=SM + 1))
            rows = ctx.enter_context(tc.tile_pool(name="rows", bufs=2))
            acc1 = ctx.enter_context(tc.tile_pool(name="acc1", bufs=2))
            ps_setup = ctx.enter_context(
                tc.tile_pool(name="ps_setup", bufs=1, space="PSUM")
            )
            ps_uk = ctx.enter_context(tc.tile_pool(name="ps_uk", bufs=3, space="PSUM"))
            ps_sc = ctx.enter_context(tc.tile_pool(name="ps_sc", bufs=2, space="PSUM"))
            ps_eb = ctx.enter_context(tc.tile_pool(name="ps_eb", bufs=2, space="PSUM"))

            # ---------------- keys pipeline helpers ----------------
            knats = {}
            pending_store = []
            pending_kts = {}

            SPC = schunk // 128

            def flush_stores():
                # emit the DRAM-scratch stores for previously loaded strips;
                # deferred a chunk behind their loads so the in-order gpsimd
                # queue never actually waits on a load completion
                while pending_store:
                    b0, c0, strips = pending_store.pop(0)
                    knat = knats[b0]
                    for i, ks in enumerate(strips):
                        si = c0 * SPC + i
                        nc.gpsimd.dma_start(
                            out=knat[si * 128 : (si + 1) * 128, :], in_=ks
                        )

            def issue_cast(b, c):
                # cast-load this chunk's keys strips f32->bf16 on gpsimd
                # (SWDGE is the only cast path); queue the staging stores
                # behind the NEXT chunk's loads
                if b not in knats:
                    knats[b] = dram_kn.tile(
                        [s, d2], bf16, tag="knat", name=f"knat_b{b}"
                    )
                strips = []
                for i in range(SPC):
                    si = c * SPC + i
                    ks = kstage.tile(
                        [128, d2], bf16, tag="ks", name=f"ks_{b}_{si}"
                    )
                    nc.gpsimd.dma_start(
                        out=ks, in_=keys_in[si * 128 : (si + 1) * 128, b, :]
                    )
                    strips.append(ks)
                flush_stores()
                pending_store.append((b, c, strips))

            def issue_ktwave(b, c):
                # one xbar wave per chunk: 16 transposed [d=128, s=schunk]
                # half-tiles into the group tile on the sync queue.  Emitted
                # right after the chunk's staging stores so the sync queue
                # only ever waits on stores that are already draining.
                g = c // LG
                sub = c % LG
                if sub == 0:
                    kts = []
                    for d in range(SD):
                        kt = ktp.tile(
                            [128, LG * schunk],
                            bf16,
                            tag="kt",
                            name=f"kt_{b}_{g}_{d}",
                        )
                        kts.append(kt)
                    pending_kts[(b, g)] = kts
                kts = pending_kts[(b, g)]
                knat = knats[b]
                for d in range(SD):
                    nc.sync.dma_start(
                        out=kts[d][:, sub * schunk : (sub + 1) * schunk],
                        in_=knat[
                            c * schunk : (c + 1) * schunk,
                            d * 128 : (d + 1) * 128,
                        ],
                        transpose=True,
                    )

            # ---------------- one-time setup ----------------
            ident_f32 = consts.tile([128, 128], fp32)
            make_identity(nc, ident_f32)

            # combined additive bias row (Wa_b + Ua_b) via accumulate-DMA;
            # issued first on gpsimd (tiny) since it gates chunk 0's tanh
            comb_bf = consts.tile([1, h], bf16)
            nc.gpsimd.dma_start(out=comb_bf, in_=uab_in)
            nc.gpsimd.dma_start(
                out=comb_bf, in_=wab_in, accum_op=ALU.add
            )

            # keys pipeline for the first chunks starts immediately
            issue_cast(0, 0)
            if NCH > 1:
                issue_cast(0, 1)
            flush_stores()
            issue_ktwave(0, 0)
            if NCH > 1:
                issue_ktwave(0, 1)
            if NCH > 2:
                issue_cast(0, 2)

            # weight panels arrive pre-transposed/pre-cast; plain HWDGE
            # loads on the scalar queue (no SWDGE cast needed)
            qT = consts.tile([128, SJ, bpc], bf16)
            nc.scalar.dma_start(out=qT, in_=qt_in)
            va_cols = consts.tile([128, SM], bf16)
            nc.scalar.dma_start(out=va_cols, in_=vac_in)
            waT = consts.tile([128, SJ, h], bf16)
            for j in range(SJ):
                nc.scalar.dma_start(
                    out=waT[:, j, :], in_=wat_in[j * 128 : (j + 1) * 128, :]
                )
            uaT = consts.tile([128, SD, h], bf16)
            for d in range(SD):
                nc.scalar.dma_start(
                    out=uaT[:, d, :], in_=uat_in[d * 128 : (d + 1) * 128, :]
                )

            ones_bf = consts.tile([1, 128], bf16)
            nc.vector.memset(ones_bf, 1.0)
            ones_f32 = consts.tile([1, 128], fp32)
            nc.vector.memset(ones_f32, 1.0)

            # bias_cols[:, m, b] = (Wa q_b)[128m:128m+128] + Wa_b + Ua_b  (fp32)
            bias_cols = consts.tile([128, SM, bpc], fp32)
            for m in range(SM):
                pw = ps_setup.tile([128, bpc], fp32, tag="setup")
                for j in range(SJ):
                    nc.tensor.matmul(
                        out=pw,
                        lhsT=waT[:, j, m * 128 : (m + 1) * 128],
                        rhs=qT[:, j, :],
                        start=(j == 0),
                        stop=False,
                    )
                nc.tensor.matmul(
                    out=pw,
                    lhsT=comb_bf[:1, m * 128 : (m + 1) * 128],
                    rhs=ones_bf[:, :bpc],
                    start=False,
                    stop=True,
                )
                nc.vector.tensor_copy(out=bias_cols[:, m, :], in_=pw)

            # ---------------- main loop over batches ----------------
            kts_group = None
            deferred_tail = []
            for b in range(bpc):
                exp_row = rows.tile([1, s], fp32, tag="exp_row", bufs=1)
                tparts = rows.tile([1, NCH], fp32, tag="tparts")
                ctxT = acc1.tile([128, SD], fp32, tag="ctxT")
                for c in range(NCH):
                    pos = b * NCH + c
                    # prefetch: strip cast-loads three chunks ahead (their
                    # stores ride one chunk behind), and the xbar wave for
                    # the chunk two ahead right after its stores are queued
                    if pos + 3 < NPOS:
                        nxt = pos + 3
                        issue_cast(nxt // NCH, nxt % NCH)
                    elif pos + 3 == NPOS:
                        flush_stores()
                    if pos + 2 < NPOS:
                        nxt = pos + 2
                        issue_ktwave(nxt // NCH, nxt % NCH)
                    if c % LG == 0:
                        kts_group = pending_kts.pop((b, c // LG))
                    kts = [
                        kt[:, (c % LG) * schunk : (c % LG + 1) * schunk]
                        for kt in kts_group
                    ]
                    if c == 1 and deferred_tail:
                        deferred_tail.pop(0)()
                    # ukT tiles + tanh; score matmuls are deferred until all
                    # tanh tiles exist so the in-order PE queue never waits
                    # on the Scalar engine mid-chunk
                    psc = ps_sc.tile([1, schunk], fp32, tag="psc")
                    ts_list = []
                    for m in range(SM):
                        puk = ps_uk.tile([128, schunk], fp32, tag="puk")
                        for d in range(SD):
                            nc.tensor.matmul(
                                out=puk,
                                lhsT=uaT[:, d, m * 128 : (m + 1) * 128],
                                rhs=kts[d],
                                start=(d == 0),
                                stop=(d == SD - 1),
                            )
                        t_sb = tp.tile([128, schunk], bf16, tag="t")
                        nc.scalar.activation(
                            out=t_sb,
                            in_=puk,
                            func=AF.Tanh,
                            bias=bias_cols[:, m, b : b + 1],
                            scale=1.0,
                        )
                        ts_list.append(t_sb)
                    for m in range(SM):
                        nc.tensor.matmul(
                            out=psc,
                            lhsT=va_cols[:, m : m + 1],
                            rhs=ts_list[m],
                            start=(m == 0),
                            stop=(m == SM - 1),
                        )
                    # exp row chunk (no max subtraction; scores are O(1)) and
                    # the chunk's softmax partial sum
                    nc.scalar.activation(
                        out=exp_row[:, c * schunk : (c + 1) * schunk],
                        in_=psc,
                        func=AF.Exp,
                        accum_out=tparts[:, c : c + 1],
                    )
                    # context on DVE: broadcast the exp row across partitions
                    # (1-row PE matmul into PSUM), then fused multiply+reduce
                    # against each transposed-keys strip, chaining the
                    # partials through the reduce's initial value
                    ebc = ps_eb.tile([128, schunk], fp32, tag="ebc")
                    nc.tensor.matmul(
                        out=ebc,
                        lhsT=ones_f32,
                        rhs=exp_row[:1, c * schunk : (c + 1) * schunk],
                        start=True,
                        stop=True,
                    )
                    sub = c % LG
                    if sub == 0:
                        ebc_sb = acc1.tile(
                            [128, LG * schunk],
                            bf16,
                            tag="ebc_sb",
                            name=f"ebcsb_{b}_{c // LG}",
                        )
                    nc.scalar.activation(
                        out=ebc_sb[:, sub * schunk : (sub + 1) * schunk],
                        in_=ebc,
                        func=AF.Copy,
                    )
                    if sub == LG - 1:
                        # per-group DVE burst: multiply each transposed-keys
                        # group tile by the exp rows and reduce along s; the
                        # mult->reduce->add chain stays on the vector queue
                        # so only the ebc copy crosses engines
                        for d in range(SD):
                            prod = acc1.tile(
                                [128, LG * schunk],
                                bf16,
                                tag="prod",
                                name=f"prod_{b}_{c}_{d}",
                            )
                            nc.vector.tensor_tensor(
                                out=prod, in0=kts_group[d], in1=ebc_sb, op=ALU.mult
                            )
                            part = acc1.tile(
                                [128, 1], fp32, tag="part", name=f"part_{b}_{c}_{d}"
                            )
                            nc.vector.reduce_sum(
                                out=part, in_=prod, axis=mybir.AxisListType.X
                            )
                            if c < LG:
                                nc.vector.tensor_copy(
                                    out=ctxT[:, d : d + 1], in_=part
                                )
                            else:
                                nc.vector.tensor_add(
                                    out=ctxT[:, d : d + 1],
                                    in0=ctxT[:, d : d + 1],
                                    in1=part,
                                )
                # softmax denominator; normalize weights, write them out
                tsum = rows.tile([1, 1], fp32, tag="tsum")
                nc.vector.reduce_sum(
                    out=tsum, in_=tparts, axis=mybir.AxisListType.X
                )
                invt = rows.tile([1, 1], fp32, tag="invt")
                nc.vector.reciprocal(out=invt, in_=tsum)
                nc.vector.tensor_scalar_mul(out=exp_row, in0=exp_row, scalar1=invt)
                nc.gpsimd.dma_start(out=w_out[b : b + 1, :], in_=exp_row)

                def ctx_tail(b=b, ctxT=ctxT, invt=invt):
                    # ctxT [d2-part, SD] -> rows [SD, 128] on PE, scaled by
                    # 1/T (replicated to SD partitions via a tiny fp32
                    # matmul).  Emitted one chunk into the NEXT batch so the
                    # in-order PE queue never waits on the final DVE context
                    # burst at the batch boundary.
                    pinv = ps_setup.tile([SD, 1], fp32, tag="setup")
                    nc.tensor.matmul(
                        out=pinv,
                        lhsT=ones_f32[:1, :SD],
                        rhs=invt,
                        start=True,
                        stop=True,
                    )
                    inv_col = rows.tile([SD, 1], fp32, tag="inv_col")
                    nc.vector.tensor_copy(out=inv_col, in_=pinv)
                    pctx = ps_setup.tile([SD, 128], fp32, tag="setup")
                    nc.tensor.transpose(
                        out=pctx, in_=ctxT, identity=ident_f32
                    )
                    ctx_rows = acc1.tile([SD, 128], fp32, tag="ctx_rows")
                    nc.vector.tensor_scalar_mul(
                        out=ctx_rows, in0=pctx, scalar1=inv_col
                    )
                    nc.gpsimd.dma_start(out=ctx_out[b : b + 1, :], in_=ctx_rows)

                deferred_tail.append(ctx_tail)
            while deferred_tail:
                deferred_tail.pop(0)()

    nc.compile()
    return nc


def _get_nc():
    if "nc" not in _CACHE:
        _CACHE["nc"] = _build()
    return _CACHE["nc"]


def _make_in_maps(inputs):
    import ml_dtypes

    bf16 = ml_dtypes.bfloat16
    SJ = H // 128
    SM = H // 128

    q_last = np.asarray(inputs["query"], dtype=np.float32)[:, -1, :]  # [B, H]
    keys = np.asarray(inputs["keys"], dtype=np.float32)  # [S, B, 2H]
    # weight panels: pre-transpose/pre-arrange + bf16 cast (layout
    # marshalling only; same rounding the device DMA cast applied)
    uaT = np.ascontiguousarray(
        np.asarray(inputs["Ua_w"], dtype=np.float32).T
    ).astype(bf16)  # [2H, H]
    waT = np.ascontiguousarray(
        np.asarray(inputs["Wa_w"], dtype=np.float32).T
    ).astype(bf16)  # [H, H]
    vac = np.ascontiguousarray(
        np.asarray(inputs["Va_w"], dtype=np.float32).reshape(SM, 128).T
    ).astype(bf16)  # [128, SM]
    uab = np.asarray(inputs["Ua_b"], dtype=np.float32).reshape(1, H).astype(bf16)
    wab = np.asarray(inputs["Wa_b"], dtype=np.float32).reshape(1, H).astype(bf16)

    in_maps = []
    for cidx in range(NCORES):
        b0 = cidx * BPC
        qt = np.ascontiguousarray(
            q_last[b0 : b0 + BPC]
            .T.reshape(SJ, 128, BPC)
            .transpose(1, 0, 2)
            .reshape(128, SJ * BPC)
        ).astype(bf16)
        in_maps.append(
            {
                "qt": qt,
                "keys": np.ascontiguousarray(keys[:, b0 : b0 + BPC, :]),
                "wat": waT,
                "uat": uaT,
                "wab": wab,
                "uab": uab,
                "vac": vac,
            }
        )
    return in_maps


def run(inputs, trace=False, **kwargs):
    """Run on all 8 cores; returns ((context, weights), BassKernelResults)."""
    from concourse.bass_utils import run_bass_kernel_spmd

    nc = _get_nc()
    in_maps = _make_in_maps(inputs)
    res = run_bass_kernel_spmd(
        nc, in_maps, core_ids=list(range(NCORES)), trace=trace, **kwargs
    )
    context = np.empty((B, 1, D2), dtype=np.float32)
    weights = np.empty((B, 1, S), dtype=np.float32)
    for c in range(NCORES):
        b0 = c * BPC
        context[b0 : b0 + BPC, 0, :] = res.results[c]["ctx"]
        weights[b0 : b0 + BPC, 0, :] = res.results[c]["wts"]
    return (context, weights), res


def kernel(**inputs):
    out, _ = run(inputs)
    return out
